# revision 1
# baseline (speedup 1.0000x reference)
"""Trainium2 Bass kernel for nn_PointTransformerLayer_78357383348685.

Reference math (B=2, N=1024, C=64):
    pos_enc = MLP(pos_i - pos_j)                     # [B,N,N,C]
    energy  = (q_i - k_i)[:, :, None, :] + pos_enc   # [B,N,N,C]
    attn    = softmax(MLP(energy), axis=-2)          # softmax over neighbor j
    out     = sum_j attn[b,i,j,c] * v[b,i,c]         # v broadcast over j (!)
    y       = relu(out @ wo + bo)

Because v is indexed by i (not j) and attn is softmaxed over j, the sum
collapses: sum_j attn[b,i,j,c] == 1, so out == v == x @ wv exactly.  The
whole N x N pairwise tensor, both MLPs, and the softmax cancel; the layer
reduces to y = relu((x @ wv) @ wo + bo).  (Verified numerically against the
reference: rel err ~2.6e-7, pure fp32 rounding noise.)

The kernel shards the B*N = 2048 rows across 8 NeuronCores (256 rows each).
Everything runs feature-major ([C, rows]) so the device does two plain
matmuls plus one fused bias+relu, with no on-device transposes; the cheap
[64, n] transposes happen host-side during shard/unshard.
"""

import numpy as np

import concourse.bacc as bacc
import concourse.mybir as mybir
import concourse.tile as tile
from concourse.bass_utils import run_bass_kernel_spmd

B, N, C = 2, 1024, 64
N_CORES = 8
ROWS = B * N                 # 2048
SHARD = ROWS // N_CORES      # 256

_CACHE = {}
LAST_RESULTS = None          # BassKernelResults of the most recent run


def _build():
    """Build + compile the per-core SPMD program.

    Inputs (per core):
      xt  [C, SHARD]  x rows for this core, feature-major
      wv  [C, C]      value projection (k-major, natural layout)
      wo  [C, C]      output projection
      bo  [C, 1]      output bias
    Output:
      out [C, SHARD]  relu((x @ wv) @ wo + bo), feature-major
    """
    f32 = mybir.dt.float32
    nc = bacc.Bacc("TRN2", target_bir_lowering=False, debug=False)

    xt = nc.dram_tensor("xt", [C, SHARD], f32, kind="ExternalInput").ap()
    wv = nc.dram_tensor("wv", [C, C], f32, kind="ExternalInput").ap()
    wo = nc.dram_tensor("wo", [C, C], f32, kind="ExternalInput").ap()
    bo = nc.dram_tensor("bo", [C, 1], f32, kind="ExternalInput").ap()
    out = nc.dram_tensor("out", [C, SHARD], f32, kind="ExternalOutput").ap()

    with tile.TileContext(nc) as tc:
        with (
            tc.tile_pool(name="sbuf", bufs=1) as pool,
            tc.tile_pool(name="psum", bufs=1, space="PSUM") as psum,
        ):
            xt_s = pool.tile([C, SHARD], f32)
            wv_s = pool.tile([C, C], f32)
            wo_s = pool.tile([C, C], f32)
            bo_s = pool.tile([C, 1], f32)
            nc.sync.dma_start(out=xt_s[:], in_=xt[:])
            nc.sync.dma_start(out=wv_s[:], in_=wv[:])
            nc.sync.dma_start(out=wo_s[:], in_=wo[:])
            nc.sync.dma_start(out=bo_s[:], in_=bo[:])

            # y1.T = (x @ wv).T = wv.T(lhsT) . xt
            y1_p = psum.tile([C, SHARD], f32)
            nc.tensor.matmul(y1_p[:], lhsT=wv_s[:], rhs=xt_s[:], start=True, stop=True)
            y1_s = pool.tile([C, SHARD], f32)
            nc.vector.tensor_copy(y1_s[:], y1_p[:])

            # y2.T = (y1 @ wo).T = wo.T(lhsT) . y1.T
            y2_p = psum.tile([C, SHARD], f32)
            nc.tensor.matmul(y2_p[:], lhsT=wo_s[:], rhs=y1_s[:], start=True, stop=True)

            # out = max(y2 + bo, 0), bo broadcast along the free (row) dim
            o_s = pool.tile([C, SHARD], f32)
            nc.vector.tensor_scalar(
                o_s[:], y2_p[:], bo_s[:], 0.0,
                mybir.AluOpType.add, mybir.AluOpType.max,
            )
            nc.sync.dma_start(out=out[:], in_=o_s[:])

    nc.compile()
    return nc


def kernel(x, pos, pe_w1, pe_b1, pe_w2, pe_b2, am_w1, am_b1, am_w2, am_b2,
           wq, wk, wv, wo, bo):
    global LAST_RESULTS
    if "nc" not in _CACHE:
        _CACHE["nc"] = _build()
    nc = _CACHE["nc"]

    xt = np.ascontiguousarray(
        np.asarray(x, dtype=np.float32).reshape(ROWS, C).T)     # [C, ROWS]
    wv = np.ascontiguousarray(np.asarray(wv, dtype=np.float32))
    wo = np.ascontiguousarray(np.asarray(wo, dtype=np.float32))
    bo = np.ascontiguousarray(np.asarray(bo, dtype=np.float32).reshape(C, 1))

    in_maps = [
        {
            "xt": np.ascontiguousarray(xt[:, i * SHARD:(i + 1) * SHARD]),
            "wv": wv,
            "wo": wo,
            "bo": bo,
        }
        for i in range(N_CORES)
    ]
    res = run_bass_kernel_spmd(nc, in_maps, list(range(N_CORES)))
    LAST_RESULTS = res
    full = np.concatenate([res.results[i]["out"] for i in range(N_CORES)], axis=1)
    return np.ascontiguousarray(full.T).reshape(B, N, C).astype(np.float32)


# revision 3
# speedup vs baseline: 1.1903x; 1.1903x over previous
"""Trainium2 Bass kernel for nn_PointTransformerLayer_78357383348685.

Reference math (B=2, N=1024, C=64):
    pos_enc = MLP(pos_i - pos_j)                     # [B,N,N,C]
    energy  = (q_i - k_i)[:, :, None, :] + pos_enc   # [B,N,N,C]
    attn    = softmax(MLP(energy), axis=-2)          # softmax over neighbor j
    out     = sum_j attn[b,i,j,c] * v[b,i,c]         # v broadcast over j (!)
    y       = relu(out @ wo + bo)

Because v is indexed by i (not j) and attn is softmaxed over j, the sum
collapses: sum_j attn[b,i,j,c] == 1, so out == v == x @ wv exactly.  The
whole N x N pairwise tensor, both MLPs, and the softmax cancel; the layer
reduces to y = relu(x @ (wv @ wo) + bo).  (Verified numerically against the
reference: rel err ~2.6e-7, pure fp32 rounding noise.)

The kernel shards the B*N = 2048 rows across 8 NeuronCores (256 rows each).
Everything runs feature-major ([C, rows]) so the device does one matmul plus
one fused bias+relu, with no on-device transposes; the cheap [64, n]
transposes and the wv@wo parameter fold happen host-side.

Device program per core (raw Bass, hand-scheduled):
    sync:   dma xt -> SBUF      (in parallel with)
    scalar: dma (W|bo) -> SBUF
    tensor: wait both; y.T = W.T(lhsT) . x.T   (PSUM)
    vector: out = max(y.T + bo, 0)             (SBUF)
    sync:   dma out -> DRAM; wait completion
"""

import numpy as np

import concourse.bacc as bacc
import concourse.mybir as mybir
from concourse.bass_utils import run_bass_kernel_spmd

B, N, C = 2, 1024, 64
N_CORES = 8
ROWS = B * N                 # 2048
SHARD = ROWS // N_CORES      # 256

_CACHE = {}
LAST_RESULTS = None          # BassKernelResults of the most recent run


def _build():
    f32 = mybir.dt.float32
    nc = bacc.Bacc("TRN2", target_bir_lowering=False, debug=False)

    xt = nc.dram_tensor("xt", [C, SHARD], f32, kind="ExternalInput").ap()
    wb = nc.dram_tensor("wb", [C, C + 1], f32, kind="ExternalInput").ap()
    out = nc.dram_tensor("out", [C, SHARD], f32, kind="ExternalOutput").ap()

    with (
        nc.sbuf_tensor("xt_s", [C, SHARD], f32) as xt_s,
        nc.sbuf_tensor("wb_s", [C, C + 1], f32) as wb_s,
        nc.sbuf_tensor("o_s", [C, SHARD], f32) as o_s,
        nc.psum_tensor("y_p", [C, SHARD], f32) as y_p,
        nc.semaphore("dsem") as dsem,
        nc.semaphore("csem") as csem,
        nc.Block() as block,
    ):
        @block.sync
        def _(sync):
            sync.dma_start(out=xt_s[:], in_=xt[:]).then_inc(dsem, 16)
            sync.wait_ge(csem, 2)
            sync.dma_start(out=out[:], in_=o_s[:]).then_inc(dsem, 16)
            sync.wait_ge(dsem, 48)

        @block.scalar
        def _(scalar):
            scalar.dma_start(out=wb_s[:], in_=wb[:]).then_inc(dsem, 16)

        @block.tensor
        def _(tensor):
            tensor.wait_ge(dsem, 32)
            tensor.matmul(
                y_p[:], lhsT=wb_s[:, 0:C], rhs=xt_s[:], start=True, stop=True,
            ).then_inc(csem, 1)

        @block.vector
        def _(vector):
            vector.wait_ge(csem, 1)
            vector.tensor_scalar(
                o_s[:], y_p[:], wb_s[:, C:C + 1], 0.0,
                mybir.AluOpType.add, mybir.AluOpType.max,
            ).then_inc(csem, 1)

    nc.compile()
    return nc


def kernel(x, pos, pe_w1, pe_b1, pe_w2, pe_b2, am_w1, am_b1, am_w2, am_b2,
           wq, wk, wv, wo, bo):
    global LAST_RESULTS
    if "nc" not in _CACHE:
        _CACHE["nc"] = _build()
    nc = _CACHE["nc"]

    xt = np.ascontiguousarray(
        np.asarray(x, dtype=np.float32).reshape(ROWS, C).T)     # [C, ROWS]
    W = np.asarray(wv, dtype=np.float32) @ np.asarray(wo, dtype=np.float32)
    wb = np.ascontiguousarray(
        np.concatenate([W, np.asarray(bo, dtype=np.float32).reshape(C, 1)],
                       axis=1))                                  # [C, C+1]

    in_maps = [
        {"xt": np.ascontiguousarray(xt[:, i * SHARD:(i + 1) * SHARD]), "wb": wb}
        for i in range(N_CORES)
    ]
    res = run_bass_kernel_spmd(nc, in_maps, list(range(N_CORES)))
    LAST_RESULTS = res
    full = np.concatenate([res.results[i]["out"] for i in range(N_CORES)], axis=1)
    return np.ascontiguousarray(full.T).reshape(B, N, C).astype(np.float32)


# revision 5
# speedup vs baseline: 1.2467x; 1.0474x over previous
"""Trainium2 Bass kernel for nn_PointTransformerLayer_78357383348685.

Reference math (B=2, N=1024, C=64):
    pos_enc = MLP(pos_i - pos_j)                     # [B,N,N,C]
    energy  = (q_i - k_i)[:, :, None, :] + pos_enc   # [B,N,N,C]
    attn    = softmax(MLP(energy), axis=-2)          # softmax over neighbor j
    out     = sum_j attn[b,i,j,c] * v[b,i,c]         # v broadcast over j (!)
    y       = relu(out @ wo + bo)

Because v is indexed by i (not j) and attn is softmaxed over j, the sum
collapses: sum_j attn[b,i,j,c] == 1, so out == v == x @ wv exactly.  The
whole N x N pairwise tensor, both MLPs, and the softmax cancel; the layer
reduces to y = relu(x @ (wv @ wo) + bo).  (Verified numerically against the
reference: rel err ~2.6e-7, pure fp32 rounding noise.)

The kernel shards the B*N = 2048 rows across 8 NeuronCores (256 rows each).
Everything runs feature-major ([C, rows]) so the device does one matmul plus
one fused bias+relu, with no on-device transposes; the cheap [64, n]
transposes and the wv@wo parameter fold happen host-side.

Device program per core (raw Bass, hand-scheduled):
    sync:   dma xt -> SBUF      (in parallel with)
    scalar: dma (W|bo) -> SBUF
    tensor: wait both; y.T = W.T(lhsT) . x.T   (PSUM)
    vector: out = max(y.T + bo, 0)             (SBUF)
    sync:   dma out -> DRAM; wait completion
"""

import numpy as np

import concourse.bacc as bacc
import concourse.mybir as mybir
from concourse.bass_utils import run_bass_kernel_spmd

B, N, C = 2, 1024, 64
N_CORES = 8
ROWS = B * N                 # 2048
SHARD = ROWS // N_CORES      # 256

_CACHE = {}
LAST_RESULTS = None          # BassKernelResults of the most recent run


def _build():
    f32 = mybir.dt.float32
    nc = bacc.Bacc("TRN2", target_bir_lowering=False, debug=False)

    xt = nc.dram_tensor("xt", [C, SHARD], f32, kind="ExternalInput").ap()
    wb = nc.dram_tensor("wb", [C, C + 1], f32, kind="ExternalInput").ap()
    out = nc.dram_tensor("out", [C, SHARD], f32, kind="ExternalOutput").ap()

    with (
        nc.sbuf_tensor("xt_s", [C, SHARD], f32) as xt_s,
        nc.sbuf_tensor("wb_s", [C, C + 1], f32) as wb_s,
        nc.sbuf_tensor("o_s", [C, SHARD], f32) as o_s,
        nc.psum_tensor("y_p", [C, SHARD], f32) as y_p,
        nc.semaphore("dsem") as dsem,
        nc.semaphore("csem") as csem,
        nc.semaphore("osem") as osem,
        nc.Block() as block,
    ):
        @block.sync
        def _(sync):
            sync.dma_start(out=xt_s[:], in_=xt[:]).then_inc(dsem, 16)
            sync.wait_ge(csem, 2)
            # No completion wait on the output DMA: the NEFF teardown's DRAIN
            # quiesces the queue, and the ~7us teardown dwarfs the transfer.
            # Completion still incs a dedicated sem no instruction waits on,
            # so repeated executions never see a stale wait condition.
            sync.dma_start(out=out[:], in_=o_s[:]).then_inc(osem, 16)

        @block.scalar
        def _(scalar):
            scalar.dma_start(out=wb_s[:], in_=wb[:]).then_inc(dsem, 16)

        @block.tensor
        def _(tensor):
            tensor.wait_ge(dsem, 32)
            tensor.matmul(
                y_p[:], lhsT=wb_s[:, 0:C], rhs=xt_s[:], start=True, stop=True,
            ).then_inc(csem, 1)

        @block.vector
        def _(vector):
            vector.wait_ge(csem, 1)
            vector.tensor_scalar(
                o_s[:], y_p[:], wb_s[:, C:C + 1], 0.0,
                mybir.AluOpType.add, mybir.AluOpType.max,
            ).then_inc(csem, 1)

    nc.compile()
    return nc


def kernel(x, pos, pe_w1, pe_b1, pe_w2, pe_b2, am_w1, am_b1, am_w2, am_b2,
           wq, wk, wv, wo, bo):
    global LAST_RESULTS
    if "nc" not in _CACHE:
        _CACHE["nc"] = _build()
    nc = _CACHE["nc"]

    xt = np.ascontiguousarray(
        np.asarray(x, dtype=np.float32).reshape(ROWS, C).T)     # [C, ROWS]
    W = np.asarray(wv, dtype=np.float32) @ np.asarray(wo, dtype=np.float32)
    wb = np.ascontiguousarray(
        np.concatenate([W, np.asarray(bo, dtype=np.float32).reshape(C, 1)],
                       axis=1))                                  # [C, C+1]

    in_maps = [
        {"xt": np.ascontiguousarray(xt[:, i * SHARD:(i + 1) * SHARD]), "wb": wb}
        for i in range(N_CORES)
    ]
    res = run_bass_kernel_spmd(nc, in_maps, list(range(N_CORES)))
    LAST_RESULTS = res
    full = np.concatenate([res.results[i]["out"] for i in range(N_CORES)], axis=1)
    return np.ascontiguousarray(full.T).reshape(B, N, C).astype(np.float32)


# revision 6
# speedup vs baseline: 1.2822x; 1.0285x over previous
"""Trainium2 Bass kernel for nn_PointTransformerLayer_78357383348685.

Reference math (B=2, N=1024, C=64):
    pos_enc = MLP(pos_i - pos_j)                     # [B,N,N,C]
    energy  = (q_i - k_i)[:, :, None, :] + pos_enc   # [B,N,N,C]
    attn    = softmax(MLP(energy), axis=-2)          # softmax over neighbor j
    out     = sum_j attn[b,i,j,c] * v[b,i,c]         # v broadcast over j (!)
    y       = relu(out @ wo + bo)

Because v is indexed by i (not j) and attn is softmaxed over j, the sum
collapses: sum_j attn[b,i,j,c] == 1, so out == v == x @ wv exactly.  The
whole N x N pairwise tensor, both MLPs, and the softmax cancel; the layer
reduces to y = relu(x @ (wv @ wo) + bo).  (Verified numerically against the
reference: rel err ~2.6e-7, pure fp32 rounding noise.)

The kernel shards the B*N = 2048 rows across 8 NeuronCores (256 rows each).
Everything runs feature-major ([C, rows]) so the device does one matmul plus
one fused bias+relu, with no on-device transposes; the cheap [64, n]
transposes and the wv@wo parameter fold happen host-side.

Device program per core (raw Bass, hand-scheduled):
    sync:   dma xt -> SBUF      (in parallel with)
    scalar: dma (W|bo) -> SBUF
    tensor: wait both; y.T = W.T(lhsT) . x.T   (PSUM)
    vector: out = max(y.T + bo, 0)             (SBUF)
    sync:   dma out -> DRAM; wait completion
"""

import numpy as np

import concourse.bacc as bacc
import concourse.mybir as mybir
from concourse.bass_utils import run_bass_kernel_spmd

B, N, C = 2, 1024, 64
N_CORES = 8
ROWS = B * N                 # 2048
SHARD = ROWS // N_CORES      # 256

_CACHE = {}
LAST_RESULTS = None          # BassKernelResults of the most recent run


def _build():
    f32 = mybir.dt.float32
    nc = bacc.Bacc("TRN2", target_bir_lowering=False, debug=False,
                   monotonic_sem_count=0)

    xt = nc.dram_tensor("xt", [C, SHARD], f32, kind="ExternalInput").ap()
    wb = nc.dram_tensor("wb", [C, C + 1], f32, kind="ExternalInput").ap()
    out = nc.dram_tensor("out", [C, SHARD], f32, kind="ExternalOutput").ap()

    with (
        nc.sbuf_tensor("xt_s", [C, SHARD], f32) as xt_s,
        nc.sbuf_tensor("wb_s", [C, C + 1], f32) as wb_s,
        nc.sbuf_tensor("o_s", [C, SHARD], f32) as o_s,
        nc.psum_tensor("y_p", [C, SHARD], f32) as y_p,
        nc.semaphore("dsem") as dsem,
        nc.semaphore("csem") as csem,
        nc.semaphore("osem") as osem,
        nc.Block() as block,
    ):
        @block.sync
        def _(sync):
            sync.dma_start(out=xt_s[:], in_=xt[:]).then_inc(dsem, 16)
            sync.wait_ge(csem, 2)
            # No completion wait on the output DMA: the NEFF teardown's DRAIN
            # quiesces the queue, and the ~7us teardown dwarfs the transfer.
            # Completion still incs a dedicated sem no instruction waits on,
            # so repeated executions never see a stale wait condition.
            sync.dma_start(out=out[:], in_=o_s[:]).then_inc(osem, 16)

        @block.scalar
        def _(scalar):
            scalar.dma_start(out=wb_s[:], in_=wb[:]).then_inc(dsem, 16)

        @block.tensor
        def _(tensor):
            tensor.wait_ge(dsem, 32)
            tensor.matmul(
                y_p[:], lhsT=wb_s[:, 0:C], rhs=xt_s[:], start=True, stop=True,
            ).then_inc(csem, 1)

        @block.vector
        def _(vector):
            vector.wait_ge(csem, 1)
            vector.tensor_scalar(
                o_s[:], y_p[:], wb_s[:, C:C + 1], 0.0,
                mybir.AluOpType.add, mybir.AluOpType.max,
            ).then_inc(csem, 1)

    nc.compile()
    return nc


def kernel(x, pos, pe_w1, pe_b1, pe_w2, pe_b2, am_w1, am_b1, am_w2, am_b2,
           wq, wk, wv, wo, bo):
    global LAST_RESULTS
    if "nc" not in _CACHE:
        _CACHE["nc"] = _build()
    nc = _CACHE["nc"]

    xt = np.ascontiguousarray(
        np.asarray(x, dtype=np.float32).reshape(ROWS, C).T)     # [C, ROWS]
    W = np.asarray(wv, dtype=np.float32) @ np.asarray(wo, dtype=np.float32)
    wb = np.ascontiguousarray(
        np.concatenate([W, np.asarray(bo, dtype=np.float32).reshape(C, 1)],
                       axis=1))                                  # [C, C+1]

    in_maps = [
        {"xt": np.ascontiguousarray(xt[:, i * SHARD:(i + 1) * SHARD]), "wb": wb}
        for i in range(N_CORES)
    ]
    res = run_bass_kernel_spmd(nc, in_maps, list(range(N_CORES)))
    LAST_RESULTS = res
    full = np.concatenate([res.results[i]["out"] for i in range(N_CORES)], axis=1)
    return np.ascontiguousarray(full.T).reshape(B, N, C).astype(np.float32)


# revision 7
# speedup vs baseline: 1.3196x; 1.0292x over previous
"""Trainium2 Bass kernel for nn_PointTransformerLayer_78357383348685.

Reference math (B=2, N=1024, C=64):
    pos_enc = MLP(pos_i - pos_j)                     # [B,N,N,C]
    energy  = (q_i - k_i)[:, :, None, :] + pos_enc   # [B,N,N,C]
    attn    = softmax(MLP(energy), axis=-2)          # softmax over neighbor j
    out     = sum_j attn[b,i,j,c] * v[b,i,c]         # v broadcast over j (!)
    y       = relu(out @ wo + bo)

Because v is indexed by i (not j) and attn is softmaxed over j, the sum
collapses: sum_j attn[b,i,j,c] == 1, so out == v == x @ wv exactly.  The
whole N x N pairwise tensor, both MLPs, and the softmax cancel; the layer
reduces to y = relu(x @ (wv @ wo) + bo).  (Verified numerically against the
reference: rel err ~2.6e-7, pure fp32 rounding noise.)

The kernel shards the B*N = 2048 rows across 8 NeuronCores (256 rows each).
Everything runs feature-major ([C, rows]) so the device does one matmul plus
one fused bias+relu, with no on-device transposes; the cheap [64, n]
transposes and the wv@wo parameter fold happen host-side.

Device program per core (raw Bass, hand-scheduled):
    sync:   dma xt -> SBUF      (in parallel with)
    scalar: dma (W|bo) -> SBUF
    tensor: wait both; y.T = W.T(lhsT) . x.T   (PSUM)
    vector: out = max(y.T + bo, 0)             (SBUF)
    sync:   dma out -> DRAM; wait completion
"""

import numpy as np

import concourse.bacc as bacc
import concourse.mybir as mybir
from concourse.bass_utils import run_bass_kernel_spmd

B, N, C = 2, 1024, 64
N_CORES = 8
ROWS = B * N                 # 2048
SHARD = ROWS // N_CORES      # 256

_CACHE = {}
LAST_RESULTS = None          # BassKernelResults of the most recent run


def _build():
    f32 = mybir.dt.float32
    nc = bacc.Bacc("TRN2", target_bir_lowering=False, debug=False,
                   monotonic_sem_count=0)

    xt = nc.dram_tensor("xt", [C, SHARD], f32, kind="ExternalInput").ap()
    wb = nc.dram_tensor("wb", [C, C + 1], f32, kind="ExternalInput").ap()
    out = nc.dram_tensor("out", [C, SHARD], f32, kind="ExternalOutput").ap()

    with (
        nc.sbuf_tensor("xt_s", [C, SHARD], f32) as xt_s,
        nc.sbuf_tensor("wb_s", [C, C + 1], f32) as wb_s,
        nc.sbuf_tensor("o_s", [C, SHARD], f32) as o_s,
        nc.psum_tensor("y_p", [C, SHARD], f32) as y_p,
        nc.semaphore("dsem") as dsem,
        nc.semaphore("csem") as csem,
        nc.semaphore("osem") as osem,
    ):
        # Block-free emission: per-engine streams come straight out of
        # program order, and skipping nc.Block() drops its end-of-block
        # all-engine barrier (redundant with the NEFF teardown's own
        # barrier) plus the per-engine branch into a sub-block.
        nc.sync.dma_start(out=xt_s[:], in_=xt[:]).then_inc(dsem, 16)
        nc.scalar.dma_start(out=wb_s[:], in_=wb[:]).then_inc(dsem, 16)

        nc.tensor.wait_ge(dsem, 32)
        nc.tensor.matmul(
            y_p[:], lhsT=wb_s[:, 0:C], rhs=xt_s[:], start=True, stop=True,
        ).then_inc(csem, 1)

        nc.vector.wait_ge(csem, 1)
        nc.vector.tensor_scalar(
            o_s[:], y_p[:], wb_s[:, C:C + 1], 0.0,
            mybir.AluOpType.add, mybir.AluOpType.max,
        ).then_inc(csem, 1)

        nc.sync.wait_ge(csem, 2)
        # No completion wait on the output DMA: the NEFF teardown's DRAIN
        # quiesces the queue, and the ~7us teardown dwarfs the transfer.
        # Completion still incs a dedicated sem no instruction waits on,
        # so repeated executions never see a stale wait condition.
        nc.sync.dma_start(out=out[:], in_=o_s[:]).then_inc(osem, 16)

    nc.compile()
    return nc


def kernel(x, pos, pe_w1, pe_b1, pe_w2, pe_b2, am_w1, am_b1, am_w2, am_b2,
           wq, wk, wv, wo, bo):
    global LAST_RESULTS
    if "nc" not in _CACHE:
        _CACHE["nc"] = _build()
    nc = _CACHE["nc"]

    xt = np.ascontiguousarray(
        np.asarray(x, dtype=np.float32).reshape(ROWS, C).T)     # [C, ROWS]
    W = np.asarray(wv, dtype=np.float32) @ np.asarray(wo, dtype=np.float32)
    wb = np.ascontiguousarray(
        np.concatenate([W, np.asarray(bo, dtype=np.float32).reshape(C, 1)],
                       axis=1))                                  # [C, C+1]

    in_maps = [
        {"xt": np.ascontiguousarray(xt[:, i * SHARD:(i + 1) * SHARD]), "wb": wb}
        for i in range(N_CORES)
    ]
    res = run_bass_kernel_spmd(nc, in_maps, list(range(N_CORES)))
    LAST_RESULTS = res
    full = np.concatenate([res.results[i]["out"] for i in range(N_CORES)], axis=1)
    return np.ascontiguousarray(full.T).reshape(B, N, C).astype(np.float32)


# revision 9
# speedup vs baseline: 1.3324x; 1.0097x over previous
"""Trainium2 Bass kernel for nn_PointTransformerLayer_78357383348685.

Reference math (B=2, N=1024, C=64):
    pos_enc = MLP(pos_i - pos_j)                     # [B,N,N,C]
    energy  = (q_i - k_i)[:, :, None, :] + pos_enc   # [B,N,N,C]
    attn    = softmax(MLP(energy), axis=-2)          # softmax over neighbor j
    out     = sum_j attn[b,i,j,c] * v[b,i,c]         # v broadcast over j (!)
    y       = relu(out @ wo + bo)

Because v is indexed by i (not j) and attn is softmaxed over j, the sum
collapses: sum_j attn[b,i,j,c] == 1, so out == v == x @ wv exactly.  The
whole N x N pairwise tensor, both MLPs, and the softmax cancel; the layer
reduces to y = relu(x @ (wv @ wo) + bo).  (Verified numerically against the
reference: rel err ~2.6e-7, pure fp32 rounding noise.)

The kernel shards the B*N = 2048 rows across 8 NeuronCores (256 rows each).
Everything runs feature-major ([C, rows]) so the device does one matmul plus
one fused bias+relu, with no on-device transposes; the cheap [64, n]
transposes and the wv@wo parameter fold happen host-side.

Device program per core (raw Bass, hand-scheduled):
    sync:   dma xt -> SBUF      (in parallel with)
    scalar: dma (W|bo) -> SBUF
    tensor: wait both; y.T = W.T(lhsT) . x.T   (PSUM)
    vector: out = max(y.T + bo, 0)             (SBUF)
    sync:   dma out -> DRAM; wait completion
"""

import os

import numpy as np

import concourse.bacc as bacc
import concourse.mybir as mybir
from concourse.bass_utils import run_bass_kernel_spmd


def _ntff_hook_available():
    """run_bass_kernel_spmd(trace=True) under axon imports antenv.axon_hooks,
    which this image may lack.  If BASS_TRACE is set in an environment without
    the hook module, the import raises and kernel() would crash — suppress
    in-library tracing in that case (external NTFF capture still works)."""
    try:
        import antenv.axon_hooks  # noqa: F401
        return True
    except ImportError:
        return False

B, N, C = 2, 1024, 64
N_CORES = 8
ROWS = B * N                 # 2048
SHARD = ROWS // N_CORES      # 256

_CACHE = {}
LAST_RESULTS = None          # BassKernelResults of the most recent run


def _build():
    f32 = mybir.dt.float32
    nc = bacc.Bacc("TRN2", target_bir_lowering=False, debug=False,
                   monotonic_sem_count=0)

    xt = nc.dram_tensor("xt", [C, SHARD], f32, kind="ExternalInput").ap()
    wb = nc.dram_tensor("wb", [C, C + 1], f32, kind="ExternalInput").ap()
    out = nc.dram_tensor("out", [C, SHARD], f32, kind="ExternalOutput").ap()

    with (
        nc.sbuf_tensor("xt_s", [C, SHARD], f32) as xt_s,
        nc.sbuf_tensor("wb_s", [C, C + 1], f32) as wb_s,
        nc.sbuf_tensor("o_s", [C, SHARD], f32) as o_s,
        nc.psum_tensor("y_p", [C, SHARD], f32) as y_p,
        nc.semaphore("dsem") as dsem,
        nc.semaphore("csem") as csem,
        nc.semaphore("osem") as osem,
    ):
        # Block-free emission: per-engine streams come straight out of
        # program order, and skipping nc.Block() drops its end-of-block
        # all-engine barrier (redundant with the NEFF teardown's own
        # barrier) plus the per-engine branch into a sub-block.
        nc.sync.dma_start(out=xt_s[:], in_=xt[:]).then_inc(dsem, 16)
        nc.scalar.dma_start(out=wb_s[:], in_=wb[:]).then_inc(dsem, 16)

        nc.tensor.wait_ge(dsem, 32)
        nc.tensor.matmul(
            y_p[:], lhsT=wb_s[:, 0:C], rhs=xt_s[:], start=True, stop=True,
        ).then_inc(csem, 1)

        nc.vector.wait_ge(csem, 1)
        nc.vector.tensor_scalar(
            o_s[:], y_p[:], wb_s[:, C:C + 1], 0.0,
            mybir.AluOpType.add, mybir.AluOpType.max,
        ).then_inc(csem, 1)

        nc.sync.wait_ge(csem, 2)
        # No completion wait on the output DMA: the NEFF teardown's DRAIN
        # quiesces the queue, and the ~7us teardown dwarfs the transfer.
        # Completion still incs a dedicated sem no instruction waits on,
        # so repeated executions never see a stale wait condition.
        nc.sync.dma_start(out=out[:], in_=o_s[:]).then_inc(osem, 16)

    nc.compile()
    return nc


def kernel(x, pos, pe_w1, pe_b1, pe_w2, pe_b2, am_w1, am_b1, am_w2, am_b2,
           wq, wk, wv, wo, bo):
    global LAST_RESULTS
    if "nc" not in _CACHE:
        _CACHE["nc"] = _build()
    nc = _CACHE["nc"]

    xt = np.ascontiguousarray(
        np.asarray(x, dtype=np.float32).reshape(ROWS, C).T)     # [C, ROWS]
    W = np.asarray(wv, dtype=np.float32) @ np.asarray(wo, dtype=np.float32)
    wb = np.ascontiguousarray(
        np.concatenate([W, np.asarray(bo, dtype=np.float32).reshape(C, 1)],
                       axis=1))                                  # [C, C+1]

    in_maps = [
        {"xt": np.ascontiguousarray(xt[:, i * SHARD:(i + 1) * SHARD]), "wb": wb}
        for i in range(N_CORES)
    ]
    guard = os.environ.get("BASS_TRACE") and not _ntff_hook_available()
    if guard:
        prev = os.environ.get("BASS_NEVER_TRACE")
        os.environ["BASS_NEVER_TRACE"] = "1"
    try:
        res = run_bass_kernel_spmd(nc, in_maps, list(range(N_CORES)))
    finally:
        if guard:
            if prev is None:
                del os.environ["BASS_NEVER_TRACE"]
            else:
                os.environ["BASS_NEVER_TRACE"] = prev
    LAST_RESULTS = res
    full = np.concatenate([res.results[i]["out"] for i in range(N_CORES)], axis=1)
    return np.ascontiguousarray(full.T).reshape(B, N, C).astype(np.float32)
